# revision 42
# baseline (speedup 1.0000x reference)
"""Single-head causal attention (B=4, S=4096, E=768, D=64) on 8 TRN2 NeuronCores.

Sharding: data-parallel over (batch, query-half): core c -> batch c//2, half c%2.
Each core receives its batch's hidden state pre-transposed to [E, S] and cast to
bf16 (host-side layout choice) and computes attention output for 2048 queries.

Per-core schedule (SPMD-uniform): queries are processed in 4 slots of 512.
Slot j covers keys [0, 1024*(j+1)).  For half p=0 the core owns the upper
512-query chunk of each 1024-block, for p=1 the lower one; the host swaps the
two 512-column halves of each 1024-key block of h^T for p=1 so both halves run
the *same* program:
  - keys [0, 1024j)            : fully unmasked
  - keys [1024j, 1024j+512)    : "dead zone" -- past for p=0 (keep), future for
                                 p=1 (killed via per-core exp bias of -80)
  - keys [1024j+512, 1024(j+1)): the core's own 512 queries -> block-triangular
                                 (0/1 multiplicative mask + memsets on exp out)

Engine budget: PE does packed [Wk|Wv] projection passes (K^T rows 0:64, V^T
rows 64:128 of one PSUM tile), Q passes, V transposes, scores, AV and the
1/den broadcast; Scalar does ONLY exp; DVE does all PSUM->SBUF drains (bias
add + bf16 cast), the triangular masks, the approx reciprocal and the final
normalize multiply.

Schedule: one flat pipeline over all (slot, super) pairs. Scores run two
supers ahead through 3 PSUM buffers (keeps the PE's LDWEIGHTS prefetch ahead
and the 2.4 GHz p-state), crossing slot boundaries. Projection octave m+1 is
interleaved into attention slot m. Diagonal supers run first in each slot
(mask chain lands mid-slot) with their causally-dead query blocks skipped in
scores/exp/AV (NARROW table).

Math: Q = h (Wq/8) + bq/8 ; scoresT[k,q] = K^T.T @ Q^T ; P = exp(scoresT)
bf16; out^T[65,512] += V_hat[k,65].T @ P where V_hat = [V | 1] so row 64
accumulates softmax denominators; final out = num * broadcast(1/den).

Known unexplored lever (~2.6us PE): interleave queries at 128-block (not
512-block) granularity with a pairwise block swap for p=1. Then odd k-tiles
carry the triangular mask at the same column for both halves (shared tri
mask) and even k-tiles need a per-core ones/zeros mask; the NARROW table
extends to the last 4 supers with qo = 128*ceil((k-1)/2), Q extraction uses
a stride-256 column AP, and the dead-zone exp bias disappears entirely.
Processed boundary columns drop from 3328 to 2560 per slot.
"""

import numpy as np
import ml_dtypes

import concourse.bass as bass
import concourse.tile as tile
from concourse import bacc, mybir
from concourse import bass_utils

B, S, E, D = 4, 4096, 768, 64
N_CORES = 8
CHUNK = 512            # queries per slot
N_SLOTS = 4            # slots per core (4 * 512 = 2048 queries)
F32 = mybir.dt.float32
BF16 = mybir.dt.bfloat16
DEAD_BIAS = -80.0


def _strided_cols(t: bass.AP, start: int, stride: int, count: int, width: int) -> bass.AP:
    """AP selecting `count` column-blocks of `width` at `start`, `start+stride`, ..."""
    base = t[:, start : start + 1]
    return bass.AP(
        tensor=base.tensor,
        offset=base.offset,
        ap=[base.ap[0], [stride, count], [1, width]],
    )


def build():
    nc = bacc.Bacc("TRN2", target_bir_lowering=False, debug=False, num_devices=N_CORES)

    ht = nc.dram_tensor("ht", [E, S], BF16, kind="ExternalInput").ap()
    # host pre-shuffles weights to partition-major so the DMA is contiguous
    wkv = nc.dram_tensor("wkv", [128, 6 * 2 * D], BF16, kind="ExternalInput").ap()
    wq = nc.dram_tensor("wq", [128, 6 * D], BF16, kind="ExternalInput").ap()
    bkv = nc.dram_tensor("bkv", [2 * D, 1], F32, kind="ExternalInput").ap()
    bq = nc.dram_tensor("bq", [D, 1], F32, kind="ExternalInput").ap()
    dead = nc.dram_tensor("dead", [128, 1], F32, kind="ExternalInput").ap()
    tri2 = nc.dram_tensor("tri2", [128, 256], BF16, kind="ExternalInput").ap()
    eye64 = nc.dram_tensor("eye64", [D, D], BF16, kind="ExternalInput").ap()
    out = nc.dram_tensor("out", [D, N_SLOTS * CHUNK], F32, kind="ExternalOutput").ap()

    n_oct = S // 1024            # 4 "octaves" of 8 key-tiles

    from contextlib import ExitStack
    with tile.TileContext(nc) as tc, ExitStack() as ctx:
        singles = ctx.enter_context(tc.tile_pool(name="singles", bufs=1))
        hpool = ctx.enter_context(tc.tile_pool(name="hpool", bufs=4))
        ptpool = ctx.enter_context(tc.tile_pool(name="ptpool", bufs=8))
        rcpool = ctx.enter_context(tc.tile_pool(name="rcpool", bufs=2))

        def _tctile(shape, dtype, _n=[0]):
            _n[0] += 1
            return singles.tile(shape, dtype, name=f"persist{_n[0]}", tag=f"persist{_n[0]}")

        # ---- persistent SBUF tensors ----
        wkv_sb = _tctile([128, 6, 2 * D], BF16)
        wq_sb = _tctile([128, 6, D], BF16)
        bkv_sb = _tctile([128, 1], F32)
        bq_sb = _tctile([D, 1], F32)
        dead_sb = _tctile([128, 1], F32)
        tri_sb = _tctile([128, 256], BF16)
        # identity content on partitions 64:128 (shares PE rows with V^T input)
        ident128 = _tctile([128, D], BF16)

        # K^T (rows 0:64) / V^T (rows 64:128) per octave, bf16
        kvoct = [_tctile([128, 1024], BF16) for _ in range(n_oct)]
        QT = _tctile([D, N_SLOTS * CHUNK], BF16)        # Q^T/8 (+bias)
        VH = _tctile([128, 32 * (D + 1)], BF16)         # V_hat tiles [128, 65] each
        out_sb = _tctile([D, N_SLOTS * CHUNK], F32)

        ones64 = _tctile([1, D], BF16)
        nc.vector.memset(ones64, 1.0)

        VH_r = VH.rearrange("p (i c) -> p i c", c=D + 1)
        nc.vector.memset(VH_r[:, :, D : D + 1], 1.0)   # ones column for denominators

        ht_r = ht.rearrange("(t p) s -> p t s", p=128)  # [128, 6, 4096]
        h_tiles = {}

        def dma_h(m):
            h = hpool.tile([128, 6, 1024], BF16, name="h", tag="h")
            # columns 512:1024 first: the q and kv-c1 projection units run
            # first and only need the second half; octave 0 is latency-bound,
            # so its leading half is further split by e-groups (the first Q
            # matmuls consume only e-chunks 0:3)
            if m == 0:
                nc.sync.dma_start(
                    out=h[:, 0:3, 512:1024],
                    in_=ht_r[:, 0:3, m * 1024 + 512 : (m + 1) * 1024],
                )
                nc.sync.dma_start(
                    out=h[:, 3:6, 512:1024],
                    in_=ht_r[:, 3:6, m * 1024 + 512 : (m + 1) * 1024],
                )
            else:
                nc.sync.dma_start(
                    out=h[:, :, 512:1024],
                    in_=ht_r[:, :, m * 1024 + 512 : (m + 1) * 1024],
                )
            nc.sync.dma_start(
                out=h[:, :, 0:512], in_=ht_r[:, :, m * 1024 : m * 1024 + 512]
            )
            h_tiles[m] = h

        with (
            tc.tile_pool(name="scps", bufs=3, space="PSUM") as scps,
            tc.tile_pool(name="ops", bufs=1, space="PSUM") as ops,
            tc.tile_pool(name="auxps", bufs=1, space="PSUM") as auxps,
        ):

            def proj_units(m):
                """Yield projection work for octave m as small closures.

                Each unit is ~2-3 PE instrs or a DVE drain; emitted interleaved
                with attention supers of slot m-1 to fill PE gaps.
                """
                state = {}

                def kv_mm(c, e0):
                    def run():
                        if e0 == 0:
                            if m == 0:
                                # prologue: scores pool is idle; avoids WAR
                                # serialization on the single aux buffer
                                s = scps.tile([128, 1024], F32, name="s", tag="s")
                                state[c] = s[:, 0:512]
                            else:
                                state[c] = auxps.tile([128, 512], F32, name="x", tag="x")
                        for e in range(e0, e0 + 3):
                            nc.tensor.matmul(
                                state[c],
                                lhsT=wkv_sb[:, e, :],
                                rhs=h_tiles[m][:, e, c * 512 : (c + 1) * 512],
                                start=e == 0, stop=e == 5, skip_group_check=True,
                            )
                    return run

                def kv_drain(c):
                    def run():
                        nc.vector.tensor_scalar_add(
                            kvoct[m][:, c * 512 : (c + 1) * 512], state[c], bkv_sb
                        )
                    return run

                def q_mm(e0):
                    def run():
                        if e0 == 0:
                            x = auxps.tile([128, 512], F32, name="x", tag="x")
                            state["q"] = x[0:D, :]
                        for e in range(e0, e0 + 3):
                            nc.tensor.matmul(
                                state["q"],
                                lhsT=wq_sb[:, e, :],
                                rhs=h_tiles[m][:, e, 512:1024],
                                start=e == 0, stop=e == 5, skip_group_check=True,
                            )
                    return run

                def q_drain():
                    nc.vector.tensor_scalar_add(
                        QT[:, m * CHUNK : (m + 1) * CHUNK], state["q"], bq_sb
                    )

                def tr_mm(t0):
                    def run():
                        if t0 == 0:
                            # PE transpose output dtype must match input (bf16):
                            # use a bf16 view of the f32 aux PSUM tile
                            x = auxps.tile([128, 512], F32, name="x", tag="x")
                            state["t"] = x.bitcast(BF16)
                        for t in range(t0, t0 + 4):
                            nc.tensor.transpose(
                                state["t"][:, t * D : (t + 1) * D],
                                kvoct[m][D : 2 * D, t * 128 : (t + 1) * 128],
                                ident128[D : 2 * D, :],
                            )
                    return run

                def tr_drain():
                    nc.vector.tensor_copy(
                        VH_r[:, m * 8 : (m + 1) * 8, 0:D],
                        state["t"][:, 0 : 8 * D].rearrange("p (i c) -> p i c", c=D),
                    )

                # q and kv-c1 first: the NEXT slot's first (diagonal)
                # scores are hoisted into this slot's tail and need QT[m]
                # and kvoct[m] columns 512:1024 already emitted
                yield q_mm(0)
                yield q_mm(3)
                yield q_drain
                yield kv_mm(1, 0)
                yield kv_mm(1, 3)
                yield kv_drain(1)
                yield kv_mm(0, 0)
                yield kv_mm(0, 3)
                yield kv_drain(0)
                yield tr_mm(0)
                yield tr_mm(4)
                yield tr_drain

            # ---- prologue: DMAs in first-use order, then octave 0 projected ----
            nc.sync.dma_start(out=wkv_sb, in_=wkv.rearrange("p (t d) -> p t d", t=6))
            dma_h(0)
            nc.sync.dma_start(out=bkv_sb, in_=bkv)
            nc.sync.dma_start(out=wq_sb, in_=wq.rearrange("p (t d) -> p t d", t=6))
            nc.sync.dma_start(out=bq_sb, in_=bq)
            nc.sync.dma_start(out=ident128[D : 2 * D, :], in_=eye64)
            nc.sync.dma_start(out=dead_sb, in_=dead)
            nc.sync.dma_start(out=tri_sb, in_=tri2)
            dma_h(1)
            for unit in proj_units(0):
                unit()

            # ---- attention slots, octave m+1 interleaved into slot m ----
            # Flat schedule over all (slot, super) pairs with a 2-deep scores
            # pipeline that crosses slot boundaries (3 scores buffers keep the
            # PE's LDWEIGHTS prefetch ahead).
            sched = []
            for j in range(N_SLOTS):
                n_sup = 4 * (j + 1)
                order = [n_sup - 2, n_sup - 1] + list(range(n_sup - 2))
                for pos, u in enumerate(order):
                    sched.append((j, u, pos, n_sup))

            slot_units = {}
            for j in range(N_SLOTS):
                units = list(proj_units(j + 1)) if j + 1 < n_oct else []
                n_sup = 4 * (j + 1)
                per_sup = [0] * n_sup
                for i in range(len(units)):
                    per_sup[i % n_sup] += 1
                slot_units[j] = (units, per_sup, [0])

            # diagonal supers (pos 0/1) skip the causally-dead query blocks:
            # pos 0 tile 1 starts at q=128; pos 1 tiles start at q=256/384
            NARROW = {(0, 1): 128, (1, 0): 256, (1, 1): 384}

            def scores(j, u, pos):
                sc_ps = scps.tile([128, 1024], F32, name="s", tag="s")
                qt_j = QT[:, j * CHUNK : (j + 1) * CHUNK]
                for t in range(2):
                    ktile = 2 * u + t
                    oct_i, kt_i = ktile // 8, ktile % 8
                    qo = NARROW.get((pos, t), 0)
                    nc.tensor.matmul(
                        sc_ps[:, t * 512 + qo : (t + 1) * 512],
                        lhsT=kvoct[oct_i][0:D, kt_i * 128 : (kt_i + 1) * 128],
                        rhs=qt_j[:, qo:CHUNK], start=True, stop=True,
                    )
                return sc_ps

            from collections import deque
            ot_tiles = {}
            sc_q = deque([scores(*sched[0][:3]), scores(*sched[1][:3])])
            for idx, (j, u, pos, n_sup) in enumerate(sched):
                if pos == 0 and j + 2 <= 3:
                    dma_h(j + 2)
                units, per_sup, ui = slot_units[j]
                pt = ptpool.tile([128, 1024], BF16, name="p", tag="p")
                bias = dead_sb if u in (n_sup - 4, n_sup - 3) else 0.0
                sc_cur = sc_q.popleft()
                if pos == 1:
                    # only the live diagonal ranges need exp
                    nc.scalar.activation(
                        pt[:, 256:512], sc_cur[:, 256:512],
                        mybir.ActivationFunctionType.Exp, bias=bias,
                    )
                    nc.scalar.activation(
                        pt[:, 896:1024], sc_cur[:, 896:1024],
                        mybir.ActivationFunctionType.Exp, bias=bias,
                    )
                else:
                    nc.scalar.activation(
                        pt, sc_cur, mybir.ActivationFunctionType.Exp, bias=bias
                    )
                if idx + 2 < len(sched):
                    sc_q.append(scores(*sched[idx + 2][:3]))
                # fill the PE pipeline while exp runs on Scalar
                for _ in range(per_sup[pos]):
                    units[ui[0]](); ui[0] += 1
                if pos == 0:
                    # diag blocks (v=0,s=0) and (v=1,s=1): cols 0 and 640
                    nc.vector.tensor_mul(
                        _strided_cols(pt, 0, 640, 2, 128),
                        _strided_cols(pt, 0, 640, 2, 128),
                        tri_sb.rearrange("p (b c) -> p b c", c=128),
                    )
                if pos == 1:
                    # diag blocks (v=2,s=2) and (v=3,s=3): cols 256 and 896
                    nc.vector.tensor_mul(
                        _strided_cols(pt, 256, 640, 2, 128),
                        _strided_cols(pt, 256, 640, 2, 128),
                        tri_sb.rearrange("p (b c) -> p b c", c=128),
                    )
                if pos == 0:
                    ot_tiles[j] = ops.tile([D + 1, CHUNK], F32, name="o", tag="o")
                ot_ps = ot_tiles[j]
                for t in range(2):
                    ktile = 2 * u + t
                    qo = NARROW.get((pos, t), 0)
                    nc.tensor.matmul(
                        ot_ps[:, qo:CHUNK],
                        lhsT=VH_r[:, ktile, :],
                        rhs=pt[:, t * 512 + qo : (t + 1) * 512],
                        start=(pos == 0 and t == 0),
                        stop=(pos == n_sup - 1 and t == 1),
                        skip_group_check=True,
                    )
                if pos == n_sup - 1:
                    assert ui[0] == len(units)
                    # normalize: out[:, q] = num[:, q] / den[q]
                    den = rcpool.tile([1, CHUNK], F32, name="den", tag="den")
                    nc.vector.tensor_copy(den, ot_ps[D : D + 1, :])
                    rc = rcpool.tile([1, CHUNK], F32, name="rc", tag="rc")
                    nc.vector.reciprocal_approx_fast(out=rc, in_=den)
                    rc_bf = rcpool.tile([1, CHUNK], BF16, name="rcb", tag="rcb")
                    nc.vector.tensor_copy(rc_bf, rc)
                    # broadcast 1/den to 64 partitions via the PE
                    r_x = auxps.tile([128, 512], F32, name="x", tag="x")
                    nc.tensor.matmul(
                        r_x[0:D, :], lhsT=ones64, rhs=rc_bf,
                        start=True, stop=True,
                    )
                    o_sl = out_sb[:, j * CHUNK : (j + 1) * CHUNK]
                    nc.vector.tensor_copy(o_sl, ot_ps[0:D, :])
                    nc.vector.tensor_mul(o_sl, o_sl, r_x[0:D, :])
                    nc.sync.dma_start(
                        out=out[:, j * CHUNK : (j + 1) * CHUNK], in_=o_sl
                    )

    nc.finalize()
    return nc


_NC_CACHE = []


def _get_nc():
    if not _NC_CACHE:
        _NC_CACHE.append(build())
    return _NC_CACHE[0]


def make_in_maps(hidden_state, Wq, bq, Wk, bk, Wv, bv):
    hidden_state = np.asarray(hidden_state, dtype=np.float32)
    tri = np.triu(np.ones((128, 128), dtype=np.float32))  # keep iff q_free >= k_part
    tri2_np = np.concatenate([tri, tri], axis=1).astype(ml_dtypes.bfloat16)
    base_w = {
        "wkv": np.ascontiguousarray(
            np.concatenate(
                [np.asarray(Wk, np.float32), np.asarray(Wv, np.float32)], axis=1
            ).astype(ml_dtypes.bfloat16)
            .reshape(6, 128, 2 * D).transpose(1, 0, 2).reshape(128, 6 * 2 * D)
        ),
        "wq": np.ascontiguousarray(
            (np.asarray(Wq, np.float32) * 0.125).astype(ml_dtypes.bfloat16)
            .reshape(6, 128, D).transpose(1, 0, 2).reshape(128, 6 * D)
        ),
        "bkv": np.ascontiguousarray(
            np.concatenate([np.asarray(bk, np.float32), np.asarray(bv, np.float32)])[
                :, None
            ]
        ),
        "bq": np.ascontiguousarray((np.asarray(bq, np.float32) * 0.125)[:, None]),
        "tri2": tri2_np,
        "eye64": np.eye(D, dtype=np.float32).astype(ml_dtypes.bfloat16),
    }
    in_maps = []
    for c in range(N_CORES):
        b, p = c // 2, c % 2
        hT = np.ascontiguousarray(hidden_state[b].T)  # [E, S]
        if p == 1:
            hT = np.ascontiguousarray(
                hT.reshape(E, S // 1024, 2, 512)[:, :, ::-1, :].reshape(E, S)
            )
        hT = hT.astype(ml_dtypes.bfloat16)
        dead_np = np.full((128, 1), DEAD_BIAS if p == 1 else 0.0, dtype=np.float32)
        in_maps.append({"ht": hT, "dead": dead_np, **base_w})
    return in_maps


def gather_output(results):
    OUT = np.empty((B, S, D), dtype=np.float32)
    for c in range(N_CORES):
        b, p = c // 2, c % 2
        o = results[c]["out"]  # [64, 2048]
        for j in range(N_SLOTS):
            chunk = 2 * j + 1 - p
            OUT[b, chunk * CHUNK : (chunk + 1) * CHUNK, :] = o[
                :, j * CHUNK : (j + 1) * CHUNK
            ].T
    return OUT


def run_cores(in_maps, **kwargs):
    nc = _get_nc()
    return bass_utils.run_bass_kernel_spmd(
        nc, in_maps, core_ids=list(range(N_CORES)), **kwargs
    )


def kernel(hidden_state, Wq, bq, Wk, bk, Wv, bv):
    in_maps = make_in_maps(hidden_state, Wq, bq, Wk, bk, Wv, bv)
    res = run_cores(in_maps)
    return gather_output(res.results)
